# revision 74
# baseline (speedup 1.0000x reference)
"""ETC transient-global self-attention on 8 TRN2 NeuronCores.

Sharding: sequence-parallel. Core c handles example b = c//4, token rows
[1024*(c%4), 1024*(c%4+1)). Each core computes its q/k/v projections (k/v
with a 1-block halo), per-example global (side) attention, and the output
projection for its token rows. No cross-core communication; the host stacks
the 8 row-slices.

Shapes (hardcoded from the problem spec):
  x  [2, 4096, 1024], Wq/Wk/Wv [1024, 16, 64], Wo [16, 64, 1024]
  block_len 128, 32 blocks, TOKENS_PER_BLOCK 16 -> G = 256 side tokens.

On-device layout notes:
  - bf16 on the PE, f32 accumulate in PSUM; bf16 output (host casts to f32).
  - QK logits are computed TRANSPOSED ([keys, q]); exp'd probs then feed PV
    as the stationary operand with out = [q, dh+1]: full 128x128 PE
    utilization and the softmax denominator (ones column of v) lands as a
    per-PARTITION column, so normalization is cheap per-partition DVE work.
  - normalized per-head-pair y tiles are transposed back to [dh, q] on the
    PE (spare region of the PV PSUM bank) for the output projection.
  - local masking: only the prev/next key blocks need masking (strict
    triangles, one shared tile, applied on the gpsimd engine);
    out-of-range halo tokens are zero-padded so their exp(0)=1 entries
    contribute 0 to numerator (v=0) and 0 to the denominator (validity
    column in v aug).
  - attention is one flat software-pipelined schedule across all 8 head
    pairs: QK runs ~3 units ahead of PV, kT projected just-in-time as
    elastic PE fill; DMs are issued smallest/earliest-needed-first so the
    PE starts at ~6us.
"""

import numpy as np
import ml_dtypes

B, L, D, H, DH = 2, 4096, 1024, 16, 64
BL = 128                 # block length
NBLK = 32                # total blocks
G = 256                  # side (global) tokens
TPB = 16                 # tokens per side block
N_CORES = 8
NB = 8                   # blocks per core
TOK = NB * BL            # 1024 q tokens per core
KV = TOK + 2 * BL        # 1280 kv tokens (1-block halo each side)
BF16 = ml_dtypes.bfloat16

_PROG = None


def _build_program():
    import concourse.bass as bass
    import concourse.mybir as mybir
    import concourse.tile as tile
    from concourse import bacc
    from concourse.masks import make_identity

    dt = mybir.dt
    f32, bf16 = dt.float32, dt.bfloat16

    nc = bacc.Bacc("TRN2", target_bir_lowering=False, debug=False,
                   num_devices=N_CORES)

    xkv_d = nc.declare_dram_parameter("xkv", [KV, D], bf16, isOutput=False)
    gT_d = nc.declare_dram_parameter("gT", [D, G], bf16, isOutput=False)
    wq_d = nc.declare_dram_parameter("wq", [D, D], bf16, isOutput=False)
    wk_d = nc.declare_dram_parameter("wk", [D, D], bf16, isOutput=False)
    wv_d = nc.declare_dram_parameter("wv", [D, D], bf16, isOutput=False)
    wo_d = nc.declare_dram_parameter("wo", [D, D], bf16, isOutput=False)
    msk_d = nc.declare_dram_parameter("maskT", [BL, 4, BL], bf16,
                                      isOutput=False)
    vld_d = nc.declare_dram_parameter("vld", [BL, 10], f32, isOutput=False)
    out_d = nc.declare_dram_parameter("out", [TOK, D], bf16, isOutput=True)

    Exp = mybir.ActivationFunctionType.Exp
    Mult = mybir.AluOpType.mult

    with tile.TileContext(nc) as tc:
        with (
            tc.tile_pool(name="per", bufs=1) as per,
            tc.tile_pool(name="wrk", bufs=2) as wrk,
            tc.tile_pool(name="obp", bufs=2) as obp,
        ):
            # ---- persistent SBUF tiles ----
            wk_sb = per.tile([128, 8, D], bf16, tag="wk")
            wv_sb = per.tile([128, 8, D], bf16, tag="wv")
            wo_sb = per.tile([128, 8, D], bf16, tag="wo")
            wq_sb = per.tile([128, 8, D], bf16, tag="wq_yT")   # dies -> yTf
            gT = per.tile([128, 8, G], bf16, tag="gT")
            skT = per.tile([128, 8, G], bf16, tag="skT")
            svaug = per.tile([128, 2, H, DH + 1], bf16, tag="svaug")
            qT = per.tile([128, 8, TOK], bf16, tag="qT")
            kT = per.tile([128, 8, KV], bf16, tag="kT")
            vaug = per.tile([128, 10, H, DH + 1], bf16, tag="vaug")
            xT = per.tile([128, 8, KV], bf16, tag="xT")
            msk2_sb = per.tile([128, 4, BL], bf16, tag="msk")
            vld_sb = per.tile([128, 10], f32, tag="vld")
            ones16 = per.tile([128, H, 1], bf16, tag="ones16")
            ident = per.tile([128, 128], f32, tag="ident")

            # DMA issue order = DMA service order (single serial resource in
            # practice): smallest/earliest-needed tensors first so the PE can
            # start projecting side-k at ~8us.
            nc.sync.dma_start(out=gT,
                              in_=gT_d.ap().rearrange("(c p) g -> p c g", p=128))
            wk_r = wk_d.ap().rearrange("(c p) d -> p c d", p=128)
            for wc in range(4):
                nc.sync.dma_start(out=wk_sb[:, :, 256 * wc:256 * (wc + 1)],
                                  in_=wk_r[:, :, 256 * wc:256 * (wc + 1)])
            nc.sync.dma_start(out=wv_sb,
                              in_=wv_d.ap().rearrange("(c p) d -> p c d", p=128))
            nc.sync.dma_start(out=msk2_sb, in_=msk_d.ap())
            nc.sync.dma_start(out=vld_sb, in_=vld_d.ap())
            for dc in range(8):
                nc.sync.dma_start(out=xT[:, dc, :],
                                  in_=xkv_d[:, dc * 128:(dc + 1) * 128],
                                  transpose=True)
            nc.sync.dma_start(out=wq_sb,
                              in_=wq_d.ap().rearrange("(c p) d -> p c d",
                                                      p=128))
            nc.sync.dma_start(out=wo_sb,
                              in_=wo_d.ap().rearrange("(c p) d -> p c d",
                                                      p=128))
            make_identity(nc, ident)
            nc.vector.memset(ones16, 1.0)

            # ---- pre-phase projections: skT, svaug, vaug, qT ----
            with tc.tile_pool(name="pp0", bufs=4, space="PSUM") as p0:
                # side kT (needs gT + wk only: starts earliest)
                for oc in range(8):
                    pp = p0.tile([128, 512], f32, tag="pj", name=f"psk{oc}")
                    for dc in range(8):
                        nc.tensor.matmul(
                            pp[:, :G],
                            wk_sb[:, dc, oc * 128:(oc + 1) * 128],
                            gT[:, dc, :],
                            start=(dc == 0), stop=(dc == 7))
                    nc.vector.tensor_copy(skT[:, oc, :], pp[:, :G])
                # side v (augmented with ones column)
                for gt_i in range(2):
                    for j in range(2):
                        pp = p0.tile([128, 512], f32, tag="pj",
                                     name=f"psv{gt_i}_{j}")
                        for dc in range(8):
                            nc.tensor.matmul(
                                pp,
                                gT[:, dc, gt_i * 128:(gt_i + 1) * 128],
                                wv_sb[:, dc, 512 * j:512 * (j + 1)],
                                start=(dc == 0), stop=(dc == 7))
                        nc.scalar.copy(
                            svaug[:, gt_i, 8 * j:8 * (j + 1), 0:DH],
                            pp.rearrange("p (h d) -> p h d", h=8))
                    nc.vector.memset(svaug[:, gt_i, :, DH:DH + 1], 1.0)
                # v natural (ones column = token validity, handles halo pads)
                for t in range(10):
                    for j in range(2):
                        pp = p0.tile([128, 512], f32, tag="pj",
                                     name=f"pv{t}_{j}")
                        for dc in range(8):
                            nc.tensor.matmul(
                                pp,
                                xT[:, dc, t * 128:(t + 1) * 128],
                                wv_sb[:, dc, 512 * j:512 * (j + 1)],
                                start=(dc == 0), stop=(dc == 7))
                        nc.scalar.copy(
                            vaug[:, t, 8 * j:8 * (j + 1), 0:DH],
                            pp.rearrange("p (h d) -> p h d", h=8))
                    nc.vector.tensor_scalar(
                        vaug[:, t, :, DH:DH + 1], ones16,
                        vld_sb[:, t:t + 1], None, Mult)
                # qT (q tokens = xT kv-rows 128..1152); Wq pre-scaled by 1/8
                for oc in range(8):
                    for tch in range(2):
                        ts_ = 128 + tch * 512
                        pp = p0.tile([128, 512], f32, tag="pj",
                                     name=f"pq{oc}_{tch}")
                        for dc in range(8):
                            nc.tensor.matmul(
                                pp,
                                wq_sb[:, dc, oc * 128:(oc + 1) * 128],
                                xT[:, dc, ts_:ts_ + 512],
                                start=(dc == 0), stop=(dc == 7))
                        nc.vector.tensor_copy(
                            qT[:, oc, tch * 512:(tch + 1) * 512], pp)

            # ---- attention: per head pair, kT projected just-in-time ----
            with tc.tile_pool(name="pkt", bufs=2, space="PSUM") as pkt, \
                 tc.tile_pool(name="plg", bufs=2, space="PSUM") as plg, \
                 tc.tile_pool(name="psg", bufs=1, space="PSUM") as psg, \
                 tc.tile_pool(name="pyt", bufs=2, space="PSUM") as pyt:
                yTf = per.tile([128, 8, TOK], bf16, tag="wq_yT")

                def kt_part(oc, ts_, te):
                    pp = pkt.tile([128, 512], f32, tag="pk",
                                  name=f"pk{oc}_{ts_}")
                    for dc in range(8):
                        nc.tensor.matmul(
                            pp[:, :te - ts_],
                            wk_sb[:, dc, oc * 128:(oc + 1) * 128],
                            xT[:, dc, ts_:te],
                            start=(dc == 0), stop=(dc == 7))
                    nc.vector.tensor_copy(kT[:, oc, ts_:te],
                                          pp[:, :te - ts_])

                def emit_us(oc, hh, nh):
                    h, r0 = 2 * oc + hh, 64 * hh
                    sg = psg.tile([128, 2, 512], f32, tag="sg",
                                  name=f"sg{h}_{nh}")
                    q4 = qT[r0:r0 + 64, oc, nh * 512:(nh + 1) * 512]
                    for g in range(2):
                        nc.tensor.matmul(
                            sg[:, g, :],
                            skT[r0:r0 + 64, oc, g * 128:(g + 1) * 128],
                            q4, start=True, stop=True)
                    u = wrk.tile([128, 2, 512], bf16, tag="us", bufs=4,
                                 name=f"us{h}_{nh}")
                    nc.scalar.activation(u, sg, Exp)
                    return u

                def emit_qk(oc, hh, n):
                    h, r0 = 2 * oc + hh, 64 * hh
                    qs = qT[r0:r0 + 64, oc, n * 128:(n + 1) * 128]
                    lg = plg.tile([128, 3, 128], f32, tag="lg",
                                  name=f"lg{h}_{n}")
                    # j-layout: [prev, next, cur] so the two masked
                    # chunks are contiguous for one gpsimd multiply.
                    for j, kv in ((0, n), (1, n + 2), (2, n + 1)):
                        nc.tensor.matmul(
                            lg[:, j, :],
                            kT[r0:r0 + 64, oc, kv * 128:(kv + 1) * 128],
                            qs, start=True, stop=True)
                    ul = wrk.tile([128, 3, 128], bf16, tag="ul", bufs=5,
                                  name=f"ul{h}_{n}")
                    nc.scalar.activation(ul, lg, Exp)
                    nc.gpsimd.tensor_mul(ul[:, 0:2, :], ul[:, 0:2, :],
                                         msk2_sb[:, 0:2, :])
                    return ul

                yb2 = {}

                def emit_pv(oc, hh, n, ul, usx):
                    h = 2 * oc + hh
                    yt = pyt.tile([128, 200], f32, tag="yt",
                                  name=f"yt{h}_{n}")
                    i4 = (n % 4) * 128
                    pairs = (
                        (ul[:, 0, :], vaug[:, n, h, :]),
                        (ul[:, 1, :], vaug[:, n + 2, h, :]),
                        (ul[:, 2, :], vaug[:, n + 1, h, :]),
                        (usx[:, 0, i4:i4 + 128], svaug[:, 0, h, :]),
                        (usx[:, 1, i4:i4 + 128], svaug[:, 1, h, :]),
                    )
                    for i, (lhs, rhs) in enumerate(pairs):
                        nc.tensor.matmul(yt[:, 0:DH + 1], lhs, rhs,
                                         start=(i == 0), stop=(i == 4))
                    rc = wrk.tile([128, 1], f32, tag="rc", bufs=4,
                                  name=f"rc{h}_{n}")
                    nc.vector.tensor_scalar_add(rc, yt[:, DH:DH + 1], 1.0)
                    rr = wrk.tile([128, 1], f32, tag="rr", bufs=4,
                                  name=f"rr{h}_{n}")
                    nc.vector.reciprocal(rr, rc)
                    if hh == 0:
                        yb2[(oc, n)] = wrk.tile([128, 128], f32, tag="yb2",
                                                bufs=2, name=f"yb{oc}_{n}")
                    nc.vector.tensor_scalar_mul(
                        yb2[(oc, n)][:, 64 * hh:64 * (hh + 1)],
                        yt[:, 0:DH], rr)
                    if hh == 1:
                        # transpose pair -> [2*dh, q] in spare psum region
                        nc.tensor.transpose(yt[:, 72:200],
                                            yb2.pop((oc, n)), ident)
                        nc.vector.tensor_copy(
                            yTf[:, oc, n * 128:(n + 1) * 128],
                            yt[:, 72:200])

                # software-pipelined schedule across ALL head pairs: QK runs
                # ~3 units ahead of PV so exp (ACT) + mask (gpsimd) latency
                # is hidden; kT projections and the next pair's prologue
                # interleave as elastic PE fill, so the pipeline never
                # drains at a head-pair boundary.
                units = [(oc, n, hh) for oc in range(8)
                         for n in range(NB) for hh in (0, 1)]
                uls = {}
                usx = {}
                qk_i, pv_i = 0, 0

                def pre_qk():
                    oc, n, hh = units[qk_i]
                    loc = qk_i - 16 * oc
                    if loc == 0:
                        kt_part(oc, 0, 512)
                        usx[(oc, 0, 0)] = emit_us(oc, 0, 0)
                    if loc == 1:
                        usx[(oc, 1, 0)] = emit_us(oc, 1, 0)
                    if loc == 4:
                        kt_part(oc, 512, 1024)
                    if loc == 9:
                        usx[(oc, 0, 1)] = emit_us(oc, 0, 1)
                    if loc == 10:
                        usx[(oc, 1, 1)] = emit_us(oc, 1, 1)
                    if (n, hh) == (6, 0):
                        kt_part(oc, 1024, 1280)

                def step_qk():
                    nonlocal qk_i
                    oc, n, hh = units[qk_i]
                    uls[(oc, n, hh)] = emit_qk(oc, hh, n)
                    qk_i += 1

                def step_pv():
                    nonlocal pv_i
                    oc, n, hh = units[pv_i]
                    emit_pv(oc, hh, n, uls.pop((oc, n, hh)),
                            usx[(oc, hh, n // 4)])
                    pv_i += 1

                for _ in range(3):
                    pre_qk()
                    step_qk()
                while pv_i < len(units):
                    if qk_i < len(units):
                        pre_qk()
                        step_qk()
                    step_pv()

            # ---- output projection ----
            with tc.tile_pool(name="pso", bufs=2, space="PSUM") as pso:
                for tt in range(8):
                    ob = obp.tile([128, D], bf16, tag="ot", name=f"ob{tt}")
                    for j in range(2):
                        pp = pso.tile([128, 512], f32, tag="po",
                                      name=f"po{tt}_{j}")
                        for oc in range(8):
                            nc.tensor.matmul(
                                pp,
                                yTf[:, oc, tt * 128:(tt + 1) * 128],
                                wo_sb[:, oc, 512 * j:512 * (j + 1)],
                                start=(oc == 0), stop=(oc == 7))
                        cp = nc.vector.tensor_copy if j == 0 else nc.scalar.copy
                        cp(ob[:, 512 * j:512 * (j + 1)], pp)
                        nc.sync.dma_start(
                            out=out_d[tt * 128:(tt + 1) * 128,
                                      512 * j:512 * (j + 1)],
                            in_=ob[:, 512 * j:512 * (j + 1)])

    nc.compile()
    return nc


def _host_inputs(x, Wq, Wk, Wv, Wo):
    """Build the 8 per-core input maps (all numpy, bf16 where device expects)."""
    xbf = x.astype(BF16)
    wq = (Wq.reshape(D, D).astype(np.float32) / np.sqrt(DH)).astype(BF16)
    wk = Wk.reshape(D, D).astype(BF16)
    wv = Wv.reshape(D, D).astype(BF16)
    wo = Wo.reshape(D, D).astype(BF16)

    # per-example side aggregates (sum of x over 16-token groups), transposed
    gT_all = np.ascontiguousarray(
        x.reshape(B, G, TPB, D).sum(2).transpose(0, 2, 1)).astype(BF16)

    # shared triangular masks [k, {prev,next}x2 heads, q]
    k_ = np.arange(BL)[:, None]
    q_ = np.arange(BL)[None, :]
    msk = np.zeros((BL, 4, BL), BF16)
    msk[:, 0, :] = msk[:, 2, :] = (k_ > q_).astype(BF16)
    msk[:, 1, :] = msk[:, 3, :] = (k_ < q_).astype(BF16)

    in_maps = []
    for c in range(N_CORES):
        b, s = c // 4, c % 4
        S0 = s * TOK
        xkv = np.zeros((KV, D), BF16)
        a0 = S0 - BL
        lo, hi = max(a0, 0), min(a0 + KV, L)
        xkv[lo - a0:hi - a0] = xbf[b, lo:hi]
        vld = np.zeros(KV, np.float32)
        vld[lo - a0:hi - a0] = 1.0
        in_maps.append({
            "xkv": xkv,
            "gT": gT_all[b],
            "wq": wq, "wk": wk, "wv": wv, "wo": wo,
            "maskT": msk,
            "vld": vld.reshape(10, BL).T.copy(),
        })
    return in_maps


_RUNNER = None


def _make_runner(nc):
    """Build the PJRT executable once; returns fn(in_maps) -> per-core outs.

    Mirrors concourse.bass2jax.run_bass_via_pjrt, but caches the jitted
    shard_map callable so repeat kernel() calls skip retrace/recompile.
    """
    import jax
    import numpy as _np
    from jax.sharding import Mesh, PartitionSpec
    from jax.experimental.shard_map import shard_map
    import concourse.mybir as mybir
    from concourse import bass2jax

    bass2jax.install_neuronx_cc_hook()
    partition_name = (nc.partition_id_tensor.name
                      if nc.partition_id_tensor else None)
    in_names, out_names, out_avals = [], [], []
    for alloc in nc.m.functions[0].allocations:
        if not isinstance(alloc, mybir.MemoryLocationSet):
            continue
        name = alloc.memorylocations[0].name
        if alloc.kind == "ExternalInput":
            if name != partition_name:
                in_names.append(name)
        elif alloc.kind == "ExternalOutput":
            out_avals.append(jax.core.ShapedArray(
                tuple(alloc.tensor_shape), mybir.dt.np(alloc.dtype)))
            out_names.append(name)
    n_params = len(in_names)
    all_names = in_names + out_names
    if partition_name is not None:
        all_names.append(partition_name)
    donate = tuple(range(n_params, n_params + len(out_names)))

    def _body(*args):
        operands = list(args)
        if partition_name is not None:
            operands.append(bass2jax.partition_id_tensor())
        return tuple(bass2jax._bass_exec_p.bind(
            *operands, out_avals=tuple(out_avals), in_names=tuple(all_names),
            out_names=tuple(out_names), lowering_input_output_aliases=(),
            sim_require_finite=True, sim_require_nnan=True, nc=nc))

    devices = jax.devices()[:N_CORES]
    mesh = Mesh(_np.asarray(devices), ("core",))
    specs = (PartitionSpec("core"),) * (n_params + len(out_names))
    sharded = jax.jit(
        shard_map(_body, mesh=mesh, in_specs=specs,
                  out_specs=(PartitionSpec("core"),) * len(out_names),
                  check_rep=False),
        donate_argnums=donate, keep_unused=True)

    def run(in_maps):
        concat_in = [
            _np.concatenate([_np.asarray(in_maps[c][k]) for c in range(N_CORES)],
                            axis=0)
            for k in in_names
        ]
        concat_zeros = [_np.zeros((N_CORES * a.shape[0], *a.shape[1:]), a.dtype)
                        for a in out_avals]
        outs = sharded(*concat_in, *concat_zeros)
        return [
            {k: _np.asarray(outs[i]).reshape(N_CORES, *out_avals[i].shape)[c]
             for i, k in enumerate(out_names)}
            for c in range(N_CORES)
        ]

    return run


def kernel(x, Wq, Wk, Wv, Wo):
    global _PROG, _RUNNER
    if _RUNNER is None:
        _PROG = _build_program()
        _RUNNER = _make_runner(_PROG)
    in_maps = _host_inputs(np.asarray(x, np.float32), np.asarray(Wq, np.float32),
                           np.asarray(Wk, np.float32), np.asarray(Wv, np.float32),
                           np.asarray(Wo, np.float32))
    results = _RUNNER(in_maps)
    out = np.empty((B, L, D), np.float32)
    for c in range(N_CORES):
        b, s = c // 4, c % 4
        out[b, s * TOK:(s + 1) * TOK] = results[c]["out"].astype(np.float32)
    return out
